# revision 8
# baseline (speedup 1.0000x reference)
"""GridMask kernel for Trainium2, 8-core data parallel — sparse quad-gather.

out[b,h,w,c] = x[b,h,w,c] * row_keep[b,h] * col_keep[b,w]

The grid mask is separable and zeroes ~50% of rows and ~50% of columns:
~75% of the output is exactly zero, and rows where row_keep==0 are zero
regardless of x. The kernel therefore only moves the surviving rows:

  - host computes the tiny per-image row/col keep vectors (exact integer
    math) and uploads x in bf16,
  - the device gathers ONLY the keep rows straight from DRAM via SWDGE
    dma_gather. Keep rows come in runs (the gaps are the zero stripes),
    so descriptors cover QUADS of 4 consecutive rows (12 KB each, run
    tails overlap backwards) — 4x fewer descriptors than row-gather,
    which matters because the Q7 descriptor-generation rate (~12 ns/desc)
    is the gather throughput ceiling,
  - the column mask is broadcast on-chip (TensorE K=1 ones-matmul into
    PSUM, ACT copies it to bf16 SBUF) and applied by DVE tensor_tensor
    at the fast 16-bit rate,
  - masked quads are stored densely packed; host scatters them into a
    zero-filled fp32 output.

Traffic is ~27% of the dense-fp32 round-trip (~7 MB/core vs 25.2 MB)
against the same 360 GB/s per-core DMA ceiling. Images are assigned to
(core, slot) by sorted quad-count so every core gathers the same padded
quad count per slot: cores stay in lockstep and padding is a few
percent (pad indices repeat the last quad; the tail is discarded on
unpack). bf16 keeps |err| <= 0.4% of |x|, well inside the 2e-2 budget.
"""

import math

import ml_dtypes
import numpy as np

import concourse.mybir as mybir
from concourse import bacc, library_config, tile
from concourse.ap import AP
from concourse.bass_utils import run_bass_kernel_spmd

B, H, W, C = 32, 512, 512, 3
D1 = 96
HH = math.ceil(math.sqrt(H * H + W * W))  # 725
OFF_H = (HH - H) // 2  # 106
OFF_W = (HH - W) // 2  # 106

NCORES = 8
BPC = B // NCORES  # images (slots) per core
FREE = W * C  # 1536 elements per image row
QR = 4  # rows per gather descriptor (quad)
QFREE = QR * FREE  # 6144 elements per quad

BF16 = mybir.dt.bfloat16
F32 = mybir.dt.float32
I16 = mybir.dt.int16

_CACHE: dict = {}


def _build_masks(d_raw, st_h_raw, st_w_raw):
    """Exact replica of the reference's integer mask math, in numpy."""
    d = D1 + d_raw.astype(np.int64)  # [B] stripe period
    l = (d + 1) // 2  # ceil(d * 0.5) for integer d
    st_h = st_h_raw.astype(np.int64) % d
    st_w = st_w_raw.astype(np.int64) % d
    yy = OFF_H + np.arange(H, dtype=np.int64)
    xx = OFF_W + np.arange(W, dtype=np.int64)
    row_zero = ((yy[None, :] - st_h[:, None]) % d[:, None]) < l[:, None]
    col_zero = ((xx[None, :] - st_w[:, None]) % d[:, None]) < l[:, None]
    return ~row_zero, ~col_zero  # [B,H], [B,W] bool


def _quads(rows):
    """Cover the sorted keep-row ids with 4-row windows.

    Returns (starts, scatter) where scatter[j] = (s, lo, hi): window j
    reads image rows [s, s+4) and rows [lo, hi) of it are real keep rows.
    Windows never cross a run gap forward; run tails overlap backwards.
    """
    starts, scat = [], []
    if len(rows) == 0:
        return starts, scat
    # split into runs
    cuts = np.nonzero(np.diff(rows) > 1)[0]
    run_bounds = np.concatenate([[0], cuts + 1, [len(rows)]])
    for i in range(len(run_bounds) - 1):
        a = int(rows[run_bounds[i]])
        b = int(rows[run_bounds[i + 1] - 1]) + 1
        s = a
        while s < b:
            if s + QR >= b:  # tail window, slide back to stay dense
                s2 = max(0, b - QR)
                starts.append(s2)
                scat.append((s2, max(a, s2), b))
                break
            starts.append(s)
            scat.append((s, s, s + QR))
            s += QR
    return starts, scat


def _build_nc(nkqs):
    """Compile the SPMD program for per-slot padded quad counts `nkqs`."""
    nc = bacc.Bacc(None)
    nrows = BPC * H  # gatherable rows per core
    sis = [(k + 15) // 16 for k in nkqs]  # idx columns per slot
    si_tot = sum(sis)
    y_len = sum(nkqs) * QFREE

    x = nc.dram_tensor("x", [nrows * FREE], BF16, kind="ExternalInput")
    idx = nc.dram_tensor("idx", [128, si_tot], I16, kind="ExternalInput")
    colm = nc.dram_tensor("colm", [1, BPC * FREE], BF16, kind="ExternalInput")
    y = nc.dram_tensor("y", [y_len], BF16, kind="ExternalOutput")

    # gather source: overlapping 4-row windows, one per row start
    x_src = AP(x, 0, [[FREE, nrows - (QR - 1)], [1, QFREE]])

    mult = mybir.AluOpType.mult
    with tile.TileContext(nc) as tc:
        with (
            tc.tile_pool(name="const", bufs=1) as cpool,
            tc.tile_pool(name="io", bufs=4) as iop,
            tc.tile_pool(name="msk", bufs=2) as mskp,
            tc.tile_pool(name="psum", bufs=2, space="PSUM") as psp,
        ):
            nc.gpsimd.load_library(library_config.mlp)
            idx_sb = cpool.tile([128, si_tot], I16, tag="idx")
            nc.sync.dma_start(idx_sb[:], idx[:])
            colm_sb = cpool.tile([1, BPC * FREE], BF16, tag="colm")
            nc.sync.dma_start(colm_sb[:], colm[:])
            ones_sb = cpool.tile([1, 128], BF16, tag="ones")
            nc.vector.memset(ones_sb[:], 1.0)

            si_off = 0
            y_off = 0
            for t in range(BPC):
                nkq = nkqs[t]
                assert nkq <= 128
                # broadcast this image's [1,1536] col mask to [128,1536]
                cmask = psp.tile([128, FREE], F32, tag="cmask")
                for ch in range(FREE // 512):
                    sl = slice(t * FREE + ch * 512, t * FREE + (ch + 1) * 512)
                    nc.tensor.matmul(
                        cmask[:, ch * 512 : (ch + 1) * 512],
                        ones_sb[:],
                        colm_sb[:, sl],
                        start=True,
                        stop=True,
                    )
                cmask_sb = mskp.tile([128, FREE], BF16, tag="cmsk")
                nc.scalar.copy(cmask_sb[:], cmask[:])

                xt = iop.tile([128, 1, QFREE], BF16, tag="xt")
                nc.gpsimd.dma_gather(
                    xt[:],
                    x_src,
                    idx_sb[:, si_off : si_off + sis[t]],
                    nkq,
                    nkq,
                    QFREE,
                    elem_step=FREE,
                )
                for q in range(QR):
                    sl = slice(q * FREE, (q + 1) * FREE)
                    nc.vector.tensor_tensor(
                        xt[:, 0, sl], xt[:, 0, sl], cmask_sb[:], op=mult
                    )
                # store the nkq quads densely packed
                nc.sync.dma_start(
                    AP(y, y_off, [[QFREE, nkq], [1, QFREE]]),
                    xt[:nkq, 0, :],
                )
                si_off += sis[t]
                y_off += nkq * QFREE
    nc.compile()
    return nc


def _prep_inputs(x, d_raw, st_h_raw, st_w_raw):
    """Compute masks, assign images to (core, slot), build per-core inputs."""
    x = np.asarray(x)
    row_keep, col_keep = _build_masks(
        np.asarray(d_raw), np.asarray(st_h_raw), np.asarray(st_w_raw)
    )
    qinfo = []  # per image: (starts, scat)
    for b in range(B):
        rows = np.nonzero(row_keep[b])[0]
        qinfo.append(_quads(rows))
    nq = np.array([len(s) for s, _ in qinfo])

    # slot-sorted assignment: slot t of core c processes image order[t*8+c]
    order = np.argsort(-nq, kind="stable")
    img_of = order.reshape(BPC, NCORES)  # [slot, core] -> image id
    nkqs = tuple(max(16, int(nq[img_of[t]].max())) for t in range(BPC))

    if _CACHE.get("nkqs") != nkqs:
        _CACHE["nc"] = _build_nc(nkqs)
        _CACHE["nkqs"] = nkqs

    x_bf = x.astype(ml_dtypes.bfloat16)  # [B,H,W,C]
    col_exp = np.repeat(col_keep, C, axis=1).astype(ml_dtypes.bfloat16)  # [B,FREE]

    sis = [(k + 15) // 16 for k in nkqs]
    si_tot = sum(sis)
    in_maps = []
    unpack = []  # per core: list of (img, scat, y_off)
    for c in range(NCORES):
        imgs = [int(img_of[t, c]) for t in range(BPC)]
        xc = x_bf[imgs].reshape(BPC * H * FREE)
        cm = col_exp[imgs].reshape(1, BPC * FREE)
        idxv = np.zeros((16, si_tot), dtype=np.int16)
        meta = []
        si_off = 0
        y_off = 0
        for t in range(BPC):
            img = imgs[t]
            starts, scat = qinfo[img]
            pad = np.zeros(sis[t] * 16, dtype=np.int16)
            if starts:
                sarr = t * H + np.asarray(starts, dtype=np.int16)
                pad[: len(sarr)] = sarr
                pad[len(sarr) : nkqs[t]] = sarr[-1]  # dup last quad
            idxv[:, si_off : si_off + sis[t]] = pad.reshape(sis[t], 16).T
            meta.append((img, scat, y_off))
            si_off += sis[t]
            y_off += nkqs[t] * QFREE
        in_maps.append({"x": xc, "idx": np.tile(idxv, (8, 1)), "colm": cm})
        unpack.append(meta)
    _CACHE["unpack"] = unpack
    return in_maps


def kernel(x, d_raw, st_h_raw, st_w_raw):
    in_maps = _prep_inputs(x, d_raw, st_h_raw, st_w_raw)
    nc = _CACHE["nc"]
    res = run_bass_kernel_spmd(nc, in_maps, list(range(NCORES)))
    out = np.zeros((B, H, W, C), dtype=np.float32)
    for c in range(NCORES):
        yc = np.asarray(res.results[c]["y"])
        for img, scat, y_off in _CACHE["unpack"][c]:
            if not scat:
                continue
            blk = yc[y_off : y_off + len(scat) * QFREE].reshape(len(scat), QR, W, C)
            # vectorized scatter: for each kept row pick (quad, row-in-quad)
            qi = np.concatenate(
                [np.full(hi - lo, j) for j, (s, lo, hi) in enumerate(scat)]
            )
            ri = np.concatenate(
                [np.arange(lo - s, hi - s) for (s, lo, hi) in scat]
            )
            dst = np.concatenate([np.arange(lo, hi) for (s, lo, hi) in scat])
            out[img, dst] = blk[qi, ri].astype(np.float32)
    return out


# revision 9
# speedup vs baseline: 1.2496x; 1.2496x over previous
"""GridMask kernel for Trainium2, 8-core data parallel — sparse row-gather.

out[b,h,w,c] = x[b,h,w,c] * row_keep[b,h] * col_keep[b,w]

The grid mask is separable and zeroes ~50% of rows and ~50% of columns:
~75% of the output is exactly zero, and rows where row_keep==0 are zero
regardless of x. The kernel therefore only moves the surviving rows:

  - host computes the tiny per-image row/col keep vectors (exact integer
    math) and uploads x in bf16,
  - the device gathers ONLY the keep rows of each image straight from
    DRAM via SWDGE dma_gather (one 3 KB row per descriptor — small
    descriptors spread across all 16 DMA engines; big fused descriptors
    land on only half of them),
  - the [128,1536] bf16 column-mask tiles arrive pre-broadcast from the
    host on the scalar HWDGE queue (overlapping the gathers) and DVE
    applies them at the 16-bit rate,
  - masked rows are stored densely packed on the sync HWDGE queue;
    host scatters them into a zero-filled fp32 output.

Traffic is ~33% of the dense-fp32 round-trip (~8.3 MB/core vs 25.2 MB)
against the same 360 GB/s per-core DMA-engine-pool ceiling. Images are
assigned to (core, slot) by sorted keep-count so every core gathers the
same padded row count per slot (pad indices repeat the last keep row;
the tail is discarded on unpack): cores stay in lockstep and padding
waste is a few percent. bf16 keeps |err| <= 0.4% of |x|, well inside
the 2e-2 relative-error budget.
"""

import math

import ml_dtypes
import numpy as np

import concourse.mybir as mybir
from concourse import bacc, library_config, tile
from concourse.ap import AP
from concourse.bass_utils import run_bass_kernel_spmd

B, H, W, C = 32, 512, 512, 3
D1 = 96
HH = math.ceil(math.sqrt(H * H + W * W))  # 725
OFF_H = (HH - H) // 2  # 106
OFF_W = (HH - W) // 2  # 106

NCORES = 8
BPC = B // NCORES  # images (slots) per core
FREE = W * C  # 1536 elements per image row

BF16 = mybir.dt.bfloat16
F32 = mybir.dt.float32
I16 = mybir.dt.int16

_CACHE: dict = {}


def _build_masks(d_raw, st_h_raw, st_w_raw):
    """Exact replica of the reference's integer mask math, in numpy."""
    d = D1 + d_raw.astype(np.int64)  # [B] stripe period
    l = (d + 1) // 2  # ceil(d * 0.5) for integer d
    st_h = st_h_raw.astype(np.int64) % d
    st_w = st_w_raw.astype(np.int64) % d
    yy = OFF_H + np.arange(H, dtype=np.int64)
    xx = OFF_W + np.arange(W, dtype=np.int64)
    row_zero = ((yy[None, :] - st_h[:, None]) % d[:, None]) < l[:, None]
    col_zero = ((xx[None, :] - st_w[:, None]) % d[:, None]) < l[:, None]
    return ~row_zero, ~col_zero  # [B,H], [B,W] bool


def _build_nc(nkps):
    """Compile the SPMD program for per-slot padded row counts `nkps`."""
    nc = bacc.Bacc(None)
    nrows = BPC * H  # gatherable rows per core
    sis = [(k + 15) // 16 for k in nkps]  # idx columns per slot
    si_tot = sum(sis)
    y_len = sum(nkps) * FREE

    x = nc.dram_tensor("x", [nrows, FREE], BF16, kind="ExternalInput")
    idx = nc.dram_tensor("idx", [128, si_tot], I16, kind="ExternalInput")
    colm = nc.dram_tensor("colm", [BPC, 128, FREE], BF16, kind="ExternalInput")
    y = nc.dram_tensor("y", [y_len], BF16, kind="ExternalOutput")

    mult = mybir.AluOpType.mult
    with tile.TileContext(nc) as tc:
        with (
            tc.tile_pool(name="const", bufs=1) as cpool,
            tc.tile_pool(name="io", bufs=4) as iop,
            tc.tile_pool(name="msk", bufs=4) as mskp,
        ):
            nc.gpsimd.load_library(library_config.mlp)
            idx_sb = cpool.tile([128, si_tot], I16, tag="idx")
            nc.scalar.dma_start(idx_sb[:], idx[:])
            cmasks = []
            for t in range(BPC):
                cm = mskp.tile([128, FREE], BF16, tag=f"cm{t}")
                nc.scalar.dma_start(cm[:], colm[t])
                cmasks.append(cm)

            si_off = 0
            y_off = 0
            for t in range(BPC):
                nkp = nkps[t]
                nb = (nkp + 127) // 128
                xt = iop.tile([128, nb, FREE], BF16, tag=f"xt{nb}")
                nc.gpsimd.dma_gather(
                    xt[:],
                    x[:],
                    idx_sb[:, si_off : si_off + sis[t]],
                    nkp,
                    nkp,
                    FREE,
                )
                for bb in range(nb):
                    nc.vector.tensor_tensor(
                        xt[:, bb, :], xt[:, bb, :], cmasks[t][:], op=mult
                    )
                # store exactly nkp rows densely: row i=(b*128+p) at y_off+1536*i
                fb, rem = divmod(nkp, 128)
                if fb:
                    nc.sync.dma_start(
                        AP(y, y_off, [[FREE, 128], [128 * FREE, fb], [1, FREE]]),
                        xt[:, :fb, :],
                    )
                if rem:
                    nc.sync.dma_start(
                        AP(y, y_off + fb * 128 * FREE, [[FREE, rem], [1, FREE]]),
                        xt[:rem, fb, :],
                    )
                si_off += sis[t]
                y_off += nkp * FREE
    nc.compile()
    return nc


def _prep_inputs(x, d_raw, st_h_raw, st_w_raw):
    """Compute masks, assign images to (core, slot), build per-core inputs."""
    x = np.asarray(x)
    row_keep, col_keep = _build_masks(
        np.asarray(d_raw), np.asarray(st_h_raw), np.asarray(st_w_raw)
    )
    nkeep = row_keep.sum(1)  # [B]

    # slot-sorted assignment: slot t of core c processes image order[t*8+c]
    order = np.argsort(-nkeep, kind="stable")
    img_of = order.reshape(BPC, NCORES)  # [slot, core] -> image id
    nkps = tuple(
        max(16, ((int(nkeep[img_of[t]].max()) + 15) // 16) * 16) for t in range(BPC)
    )

    if _CACHE.get("nkps") != nkps:
        _CACHE["nc"] = _build_nc(nkps)
        _CACHE["nkps"] = nkps

    x_bf = x.astype(ml_dtypes.bfloat16)  # [B,H,W,C]
    col_exp = np.repeat(col_keep, C, axis=1).astype(ml_dtypes.bfloat16)  # [B,FREE]

    sis = [(k + 15) // 16 for k in nkps]
    si_tot = sum(sis)
    in_maps = []
    unpack = []  # per core: list of (img, rows, y_off, nkeep)
    for c in range(NCORES):
        imgs = [int(img_of[t, c]) for t in range(BPC)]
        xc = x_bf[imgs].reshape(BPC * H, FREE)
        cm = np.ascontiguousarray(
            np.broadcast_to(col_exp[imgs][:, None, :], (BPC, 128, FREE))
        )
        idxv = np.zeros((16, si_tot), dtype=np.int16)
        meta = []
        si_off = 0
        y_off = 0
        for t in range(BPC):
            img = imgs[t]
            rows = np.nonzero(row_keep[img])[0].astype(np.int16)
            nk = len(rows)
            pad = np.zeros(sis[t] * 16, dtype=np.int16)
            if nk:
                pad[:nk] = t * H + rows
                pad[nk : nkps[t]] = pad[nk - 1]  # dup last keep row
            idxv[:, si_off : si_off + sis[t]] = pad.reshape(sis[t], 16).T
            meta.append((img, rows, y_off, nk))
            si_off += sis[t]
            y_off += nkps[t] * FREE
        in_maps.append({"x": xc, "idx": np.tile(idxv, (8, 1)), "colm": cm})
        unpack.append(meta)
    _CACHE["unpack"] = unpack
    return in_maps


def kernel(x, d_raw, st_h_raw, st_w_raw):
    in_maps = _prep_inputs(x, d_raw, st_h_raw, st_w_raw)
    nc = _CACHE["nc"]
    res = run_bass_kernel_spmd(nc, in_maps, list(range(NCORES)))
    out = np.zeros((B, H, W, C), dtype=np.float32)
    for c in range(NCORES):
        yc = np.asarray(res.results[c]["y"])
        for img, rows, y_off, nk in _CACHE["unpack"][c]:
            if nk:
                blk = yc[y_off : y_off + nk * FREE].reshape(nk, W, C)
                out[img, rows] = blk.astype(np.float32)
    return out


# revision 10
# speedup vs baseline: 1.5325x; 1.2264x over previous
"""GridMask kernel for Trainium2, 8-core data parallel — sparse row-gather.

out[b,h,w,c] = x[b,h,w,c] * row_keep[b,h] * col_keep[b,w]

The grid mask is separable and zeroes ~50% of rows and ~50% of columns:
~75% of the output is exactly zero, and rows where row_keep==0 are zero
regardless of x. The kernel therefore only moves the surviving rows:

  - host computes the tiny per-image row/col keep vectors (exact integer
    math) and uploads x in bf16,
  - the device gathers ONLY the keep rows of each image straight from
    DRAM via SWDGE dma_gather (one 3 KB row per descriptor — small
    descriptors spread across all 16 DMA engines),
  - the [1,1536] col masks are broadcast on-chip (TensorE K=1 ones
    matmul into PSUM, ACT stages them to bf16 SBUF) and applied by DVE
    tensor_tensor at the 16-bit rate,
  - masked rows are stored densely packed on the sync HWDGE queue;
    host scatters them into a zero-filled fp32 output.

All four gathers are emitted immediately after the index upload: tile
DMA semaphores are cumulative per queue, so anything enqueued earlier
on the same queue (weight loads, mask loads) would gate the first
gather by several microseconds.

Traffic is ~27% of the dense-fp32 round-trip (~6.7 MB/core vs 25.2 MB)
against the same 360 GB/s per-core DMA-engine-pool ceiling. Images are
assigned to (core, slot) by sorted keep-count so every core gathers the
same padded row count per slot (pad indices repeat the last keep row;
the tail is discarded on unpack): cores stay in lockstep and padding
waste is a few percent. bf16 keeps |err| <= 0.4% of |x|, well inside
the 2e-2 relative-error budget.
"""

import math

import ml_dtypes
import numpy as np

import concourse.mybir as mybir
from concourse import bacc, library_config, tile
from concourse.ap import AP
from concourse.bass_utils import run_bass_kernel_spmd

B, H, W, C = 32, 512, 512, 3
D1 = 96
HH = math.ceil(math.sqrt(H * H + W * W))  # 725
OFF_H = (HH - H) // 2  # 106
OFF_W = (HH - W) // 2  # 106

NCORES = 8
BPC = B // NCORES  # images (slots) per core
FREE = W * C  # 1536 elements per image row

BF16 = mybir.dt.bfloat16
F32 = mybir.dt.float32
I16 = mybir.dt.int16

_CACHE: dict = {}


def _build_masks(d_raw, st_h_raw, st_w_raw):
    """Exact replica of the reference's integer mask math, in numpy."""
    d = D1 + d_raw.astype(np.int64)  # [B] stripe period
    l = (d + 1) // 2  # ceil(d * 0.5) for integer d
    st_h = st_h_raw.astype(np.int64) % d
    st_w = st_w_raw.astype(np.int64) % d
    yy = OFF_H + np.arange(H, dtype=np.int64)
    xx = OFF_W + np.arange(W, dtype=np.int64)
    row_zero = ((yy[None, :] - st_h[:, None]) % d[:, None]) < l[:, None]
    col_zero = ((xx[None, :] - st_w[:, None]) % d[:, None]) < l[:, None]
    return ~row_zero, ~col_zero  # [B,H], [B,W] bool


def _build_nc(nkps):
    """Compile the SPMD program for per-slot padded row counts `nkps`."""
    nc = bacc.Bacc(None)
    nrows = BPC * H  # gatherable rows per core
    sis = [(k + 15) // 16 for k in nkps]  # idx columns per slot
    si_tot = sum(sis)
    y_len = sum(nkps) * FREE

    x = nc.dram_tensor("x", [nrows, FREE], BF16, kind="ExternalInput")
    idx = nc.dram_tensor("idx", [128, si_tot], I16, kind="ExternalInput")
    colm = nc.dram_tensor("colm", [1, BPC * FREE], BF16, kind="ExternalInput")
    y = nc.dram_tensor("y", [y_len], BF16, kind="ExternalOutput")

    mult = mybir.AluOpType.mult
    with tile.TileContext(nc) as tc:
        with (
            tc.tile_pool(name="const", bufs=1) as cpool,
            tc.tile_pool(name="io", bufs=4) as iop,
            tc.tile_pool(name="msk", bufs=4) as mskp,
            tc.tile_pool(name="psum", bufs=2, space="PSUM") as psp,
        ):
            nc.gpsimd.load_library(library_config.mlp)
            idx_sb = cpool.tile([128, si_tot], I16, tag="idx")
            nc.scalar.dma_start(idx_sb[:], idx[:])

            # all gathers first: queue DMA semaphores are cumulative, so
            # these must precede every other DMA/weight-load emission.
            xts = []
            si_off = 0
            for t in range(BPC):
                nkp = nkps[t]
                nb = (nkp + 127) // 128
                xt = iop.tile([128, nb, FREE], BF16, tag=f"xt{nb}")
                nc.gpsimd.dma_gather(
                    xt[:],
                    x[:],
                    idx_sb[:, si_off : si_off + sis[t]],
                    nkp,
                    nkp,
                    FREE,
                )
                xts.append(xt)
                si_off += sis[t]

            colm_sb = cpool.tile([1, BPC * FREE], BF16, tag="colm")
            nc.scalar.dma_start(colm_sb[:], colm[:])
            ones_sb = cpool.tile([1, 128], BF16, tag="ones")
            nc.vector.memset(ones_sb[:], 1.0)

            y_off = 0
            for t in range(BPC):
                nkp = nkps[t]
                nb = (nkp + 127) // 128
                xt = xts[t]
                # broadcast this image's [1,1536] col mask to [128,1536]
                cmask = psp.tile([128, FREE], F32, tag="cmask")
                for ch in range(FREE // 512):
                    sl = slice(t * FREE + ch * 512, t * FREE + (ch + 1) * 512)
                    nc.tensor.matmul(
                        cmask[:, ch * 512 : (ch + 1) * 512],
                        ones_sb[:],
                        colm_sb[:, sl],
                        start=True,
                        stop=True,
                    )
                # stage to bf16 SBUF so DVE multiplies hit the 16-bit rate
                cmask_sb = mskp.tile([128, FREE], BF16, tag="cmsk")
                nc.scalar.copy(cmask_sb[:], cmask[:])
                for bb in range(nb):
                    nc.vector.tensor_tensor(
                        xt[:, bb, :], xt[:, bb, :], cmask_sb[:], op=mult
                    )
                # store exactly nkp rows densely: row i=(b*128+p) at y_off+1536*i
                fb, rem = divmod(nkp, 128)
                if fb:
                    nc.sync.dma_start(
                        AP(y, y_off, [[FREE, 128], [128 * FREE, fb], [1, FREE]]),
                        xt[:, :fb, :],
                    )
                if rem:
                    nc.sync.dma_start(
                        AP(y, y_off + fb * 128 * FREE, [[FREE, rem], [1, FREE]]),
                        xt[:rem, fb, :],
                    )
                y_off += nkp * FREE
    nc.compile()
    return nc


def _prep_inputs(x, d_raw, st_h_raw, st_w_raw):
    """Compute masks, assign images to (core, slot), build per-core inputs."""
    x = np.asarray(x)
    row_keep, col_keep = _build_masks(
        np.asarray(d_raw), np.asarray(st_h_raw), np.asarray(st_w_raw)
    )
    nkeep = row_keep.sum(1)  # [B]

    # slot-sorted assignment: slot t of core c processes image order[t*8+c]
    order = np.argsort(-nkeep, kind="stable")
    img_of = order.reshape(BPC, NCORES)  # [slot, core] -> image id
    nkps = tuple(
        max(16, ((int(nkeep[img_of[t]].max()) + 15) // 16) * 16) for t in range(BPC)
    )

    if _CACHE.get("nkps") != nkps:
        _CACHE["nc"] = _build_nc(nkps)
        _CACHE["nkps"] = nkps

    x_bf = x.astype(ml_dtypes.bfloat16)  # [B,H,W,C]
    col_exp = np.repeat(col_keep, C, axis=1).astype(ml_dtypes.bfloat16)  # [B,FREE]

    sis = [(k + 15) // 16 for k in nkps]
    si_tot = sum(sis)
    in_maps = []
    unpack = []  # per core: list of (img, rows, y_off, nkeep)
    for c in range(NCORES):
        imgs = [int(img_of[t, c]) for t in range(BPC)]
        xc = x_bf[imgs].reshape(BPC * H, FREE)
        cm = col_exp[imgs].reshape(1, BPC * FREE)
        idxv = np.zeros((16, si_tot), dtype=np.int16)
        meta = []
        si_off = 0
        y_off = 0
        for t in range(BPC):
            img = imgs[t]
            rows = np.nonzero(row_keep[img])[0].astype(np.int16)
            nk = len(rows)
            pad = np.zeros(sis[t] * 16, dtype=np.int16)
            if nk:
                pad[:nk] = t * H + rows
                pad[nk : nkps[t]] = pad[nk - 1]  # dup last keep row
            idxv[:, si_off : si_off + sis[t]] = pad.reshape(sis[t], 16).T
            meta.append((img, rows, y_off, nk))
            si_off += sis[t]
            y_off += nkps[t] * FREE
        in_maps.append({"x": xc, "idx": np.tile(idxv, (8, 1)), "colm": cm})
        unpack.append(meta)
    _CACHE["unpack"] = unpack
    return in_maps


def kernel(x, d_raw, st_h_raw, st_w_raw):
    in_maps = _prep_inputs(x, d_raw, st_h_raw, st_w_raw)
    nc = _CACHE["nc"]
    res = run_bass_kernel_spmd(nc, in_maps, list(range(NCORES)))
    out = np.zeros((B, H, W, C), dtype=np.float32)
    for c in range(NCORES):
        yc = np.asarray(res.results[c]["y"])
        for img, rows, y_off, nk in _CACHE["unpack"][c]:
            if nk:
                blk = yc[y_off : y_off + nk * FREE].reshape(nk, W, C)
                out[img, rows] = blk.astype(np.float32)
    return out
